# revision 23
# baseline (speedup 1.0000x reference)
"""Trainium2 Bass kernel for nn_Attention_41996190220419.

Single-head causal attention with softplus weights and a time-flipped
rotary embedding, B=8 T=2048 C=1024 fp32.

Sharding: pure data-parallel over batch (1 batch element per NeuronCore,
8 cores, no collectives).

v2 over the 420us baseline:
  - K/Q projection GEMMs and the score GEMM run in fp8 e4m3 with
    MatmulPerfMode.DoubleRow (2 fp8 weights per PE cell, K=256 per
    instruction, ~1.4x bf16 throughput).  The V/AV/proj GEMMs stay
    16-bit: fp8 anywhere in the value path puts ~2.4% quantization
    error straight onto the output (softplus smoothing only protects
    the score path).  Simulated end-to-end L2 err 1.1e-2 vs 2e-2 gate.
  - x arrives host-pretransposed (fp16 for V, fp8 pairs for K/Q), so
    the PE transpose phase is gone.
  - 16-bit GEMMs use fp16 instead of bf16 (same PE speed, 8x lower
    rounding error).
  - rotary cos/sin tables are built from the fp32-rounded angle t*j the
    reference computes, eliminating a systematic trig mismatch.

Per-core phases:
  1. KT/QT = W^T x^T via fp8-DR (+bias via ACT), rotary rotation on DVE
     (fp16 cos/sin streamed), results quantized to fp8 pair tiles.
  2. V = x Wv + bv into resident fp16 SBUF tiles (overlaps the DVE
     rotation tail).
  3. per 512-wide i-span: ST[j,i] = sum_c QR[j,c] KR[i,c] (fp8-DR) for
     causal blocks, softplus = Ln(Exp(x)+1) on ACT, diagonal masks on
     DVE; OT[c,i] = sum_j V[j,c] ST[j,i] (fp16); OUT = OT^T Wp + bp.

The even/odd rotation pairs are turned into tile-level structure by
permuting the columns of Wk/Wq (and bk/bq) on the host to [evens|odds];
scores are invariant to any channel permutation applied to both K and Q.
"""

import os
import sys

if "/opt/trn_rl_repo" not in sys.path:
    sys.path.insert(0, "/opt/trn_rl_repo")

import numpy as np
import ml_dtypes

import concourse.bass as bass
import concourse.bacc as bacc
import concourse.mybir as mybir
import concourse.tile as tile
from concourse.bass_utils import run_bass_kernel_spmd

B, T, C = 8, 2048, 1024
H = C // 2
NCORES = 8
PD = 128
TCH = 512                 # t-chunk width (phase 1) == i-span width (attention)
NT = T // PD              # 16
NSP = T // TCH            # 4
NG = C // PD              # 8
NP = NG // 2              # 4 c-group pairs for DoubleRow
F16 = mybir.dt.float16
FP8 = mybir.dt.float8e4
F32 = mybir.dt.float32
AF = mybir.ActivationFunctionType
DR = mybir.MatmulPerfMode.DoubleRow
INV_SQRT_C = float(C) ** -0.5

_CACHE = {}

LAST_RESULT = None  # BassKernelResults of the most recent run (for profiling)


def _patch_act_tables():
    """Force every ACT func we use (Copy/Identity/Exp/Ln) to resolve to the
    single `natural_log_exp_and_others` table so the Exp/Ln alternation in
    the softplus does not thrash ACT_TABLE_LOADs (1.3us each)."""
    if _CACHE.get("act_patched"):
        return
    from concourse import hw_specs
    orig = hw_specs.get_activation_tables
    combined = "natural_log_exp_and_others"

    def patched(arch):
        tables = orig(arch)
        if combined in tables:
            keep = tables[combined]
            tables = {
                name: (s if name == combined else (s - keep))
                for name, s in tables.items()
            }
        return tables

    hw_specs.get_activation_tables = patched
    bacc.get_activation_tables = patched
    _CACHE["act_patched"] = True


def _build_nc():
    _patch_act_tables()
    nc = bacc.Bacc("TRN2", target_bir_lowering=False, debug=False,
                   num_devices=NCORES)

    xt16_d = nc.dram_tensor("xt16", [NG, PD, T], F16, kind="ExternalInput").ap()
    xt8_d = nc.dram_tensor("xt8", [NSP, PD, NP, 2, TCH], FP8,
                           kind="ExternalInput").ap()
    # g-axis pre-ordered [0,4,1,5,2,6,3,7] so slot pair 2e..2e+1 = groups e,e+4
    wk8_d = nc.dram_tensor("wk8", [4, PD, 2, NG, PD], FP8, kind="ExternalInput").ap()
    wq8_d = nc.dram_tensor("wq8", [4, PD, 2, NG, PD], FP8, kind="ExternalInput").ap()
    wv_d = nc.dram_tensor("wv", [NG, PD, C], F16, kind="ExternalInput").ap()
    wp_d = nc.dram_tensor("wp", [NG, PD, C], F16, kind="ExternalInput").ap()
    bkr_d = nc.dram_tensor("bkr", [PD, NG], F32, kind="ExternalInput").ap()
    bqr_d = nc.dram_tensor("bqr", [PD, NG], F32, kind="ExternalInput").ap()
    bvb_d = nc.dram_tensor("bvb", [PD, C], F32, kind="ExternalInput").ap()
    cos_d = nc.dram_tensor("cosT", [H, T], F16, kind="ExternalInput").ap()
    sin_d = nc.dram_tensor("sinT", [H, T], F16, kind="ExternalInput").ap()
    msk_d = nc.dram_tensor("masks", [NSP, PD, TCH], F16,
                           kind="ExternalInput").ap()
    out_d = nc.dram_tensor("out", [T, C], F32, kind="ExternalOutput").ap()

    with tile.TileContext(nc) as tc:
        with tc.tile_pool(name="persist", bufs=1) as pp, \
             tc.tile_pool(name="psA", bufs=4, space="PSUM") as psA, \
             tc.tile_pool(name="psB", bufs=4, space="PSUM") as psB:

            # fp8 rotated K/Q: pair tile P holds c-groups (2P, 2P+1)
            kr8 = [pp.tile([PD, 2, T], FP8, tag=f"kr{p}", name=f"kr{p}")
                   for p in range(NP)]
            qr8 = [pp.tile([PD, 2, T], FP8, tag=f"qr{p}", name=f"qr{p}")
                   for p in range(NP)]
            vsb = [pp.tile([PD, C], F16, tag=f"v{j}", name=f"v{j}")
                   for j in range(NT)]

            bkr = pp.tile([PD, NG], F32, name="bkr")
            nc.sync.dma_start(out=bkr, in_=bkr_d)
            bqr = pp.tile([PD, NG], F32, name="bqr")
            nc.sync.dma_start(out=bqr, in_=bqr_d)
            # bvb/masks DMAs are issued later (not needed until V GEMM /
            # attention) to keep the startup sync queue lean
            bvb = pp.tile([PD, C], F32, name="bvb")
            mskt = [pp.tile([PD, TCH], F16, tag=f"msk{d}", name=f"msk{d}")
                    for d in range(NSP)]

            # ---------------- phase 1+2: K/Q (rotated, fp8) then V --------
            with tc.tile_pool(name="p1", bufs=1) as p1:
                # DMA emission order == sync-queue order: first-needed first.
                xt8 = p1.tile([PD, NSP, NP, 2, TCH], FP8, name="xt8")
                wk8 = p1.tile([PD, 8, NG, PD], FP8, name="wk8")
                wq8 = p1.tile([PD, 8, NG, PD], FP8, name="wq8")
                nc.sync.dma_start(out=xt8[:, 0], in_=xt8_d[0])
                nc.sync.dma_start(out=wk8[:, 0:2], in_=wk8_d[0])
                for ch in range(1, NSP):
                    nc.sync.dma_start(out=xt8[:, ch], in_=xt8_d[ch])
                for e in range(1, 4):
                    nc.sync.dma_start(out=wk8[:, 2 * e:2 * e + 2],
                                      in_=wk8_d[e])

                def kq_gemms(wname, w8t, brt, dst):
                    # weight slot for group g: pairs (e, e+4) live at 2e, 2e+1
                    for e in range(4):
                        trig = {}
                        for ch in range(NSP):
                            csl = slice(ch * TCH, (ch + 1) * TCH)
                            cs = p1.tile([PD, TCH], F16, tag="trig", bufs=16,
                                         name=f"cs{wname}{e}_{ch}")
                            nc.sync.dma_start(
                                out=cs, in_=cos_d[e * PD:(e + 1) * PD, csl])
                            sn = p1.tile([PD, TCH], F16, tag="trig", bufs=16,
                                         name=f"sn{wname}{e}_{ch}")
                            nc.sync.dma_start(
                                out=sn, in_=sin_d[e * PD:(e + 1) * PD, csl])
                            trig[ch] = (cs, sn)
                        for ch in range(NSP):
                            csl = slice(ch * TCH, (ch + 1) * TCH)
                            tmp = {}
                            for i, g in enumerate((e, e + 4)):
                                ps = psA.tile([PD, TCH], F32, tag="ps_mm",
                                              name=f"pkq{wname}{g}_{ch}")
                                for m in range(NP):
                                    nc.tensor.matmul(
                                        ps,
                                        lhsT=w8t[:, 2 * e + i, 2 * m:2 * m + 2, :],
                                        rhs=xt8[:, ch, m],
                                        start=(m == 0), stop=(m == NP - 1),
                                        perf_mode=DR)
                                kt = p1.tile([PD, TCH], F16, tag="kttmp",
                                             bufs=8, name=f"kt{wname}{g}_{ch}")
                                nc.scalar.activation(kt, ps, AF.Identity,
                                                     bias=brt[:, g:g + 1])
                                tmp[g] = kt
                            cs, sn = trig[ch]
                            ze, zo = tmp[e], tmp[e + 4]
                            # r_even = cos*z_e + sin*z_o ; r_odd = cos*z_o - sin*z_e
                            # muls for the odd half run on GPSIMD to split the
                            # elementwise load across engines
                            t1 = p1.tile([PD, TCH], F16, tag="rot", bufs=6,
                                         name=f"r1{wname}{e}_{ch}")
                            nc.vector.tensor_mul(t1, ze, cs)
                            t2 = p1.tile([PD, TCH], F16, tag="rot", bufs=6,
                                         name=f"r2{wname}{e}_{ch}")
                            nc.vector.tensor_mul(t2, zo, sn)
                            nc.vector.tensor_add(
                                dst[e // 2][:, e % 2, csl], t1, t2)
                            t3 = p1.tile([PD, TCH], F16, tag="rotg", bufs=6,
                                         name=f"r3{wname}{e}_{ch}")
                            nc.gpsimd.tensor_mul(t3, zo, cs)
                            t4 = p1.tile([PD, TCH], F16, tag="rotg", bufs=6,
                                         name=f"r4{wname}{e}_{ch}")
                            nc.gpsimd.tensor_mul(t4, ze, sn)
                            nc.vector.tensor_sub(
                                dst[2 + e // 2][:, e % 2, csl], t3, t4)

                kq_gemms("k", wk8, bkr, kr8)

                # V-GEMM operands + Q weights stream behind the K-phase DMAs
                xt16 = []
                for g in range(NG):
                    t16 = p1.tile([PD, T], F16, tag=f"xt16_{g}",
                                  name=f"xt16_{g}")
                    nc.sync.dma_start(out=t16, in_=xt16_d[g])
                    xt16.append(t16)
                wvsb = []
                for g in range(NG):
                    wt = p1.tile([PD, C], F16, tag="wv", bufs=8,
                                 name=f"wv{g}")
                    nc.sync.dma_start(out=wt, in_=wv_d[g])
                    wvsb.append(wt)
                for e in range(4):
                    nc.sync.dma_start(out=wq8[:, 2 * e:2 * e + 2],
                                      in_=wq8_d[e])

                nc.sync.dma_start(out=bvb, in_=bvb_d)
                for d in range(NSP):
                    nc.sync.dma_start(out=mskt[d], in_=msk_d[d])

                kq_gemms("q", wq8, bqr, qr8)

                # V GEMM (fp16): writes into resident vsb tiles
                for tt in range(NT):
                    for h in range(2):
                        ps = psB.tile([PD, TCH], F32, tag="ps_ot",
                                      name=f"pv{tt}_{h}")
                        for g in range(NG):
                            nc.tensor.matmul(
                                ps,
                                lhsT=xt16[g][:, tt * PD:(tt + 1) * PD],
                                rhs=wvsb[g][:, h * TCH:(h + 1) * TCH],
                                start=(g == 0), stop=(g == NG - 1))
                        nc.vector.tensor_add(vsb[tt][:, h * TCH:(h + 1) * TCH],
                                             ps, bvb[:, h * TCH:(h + 1) * TCH])

            # ---------------- phase 3: attention + projection -------------
            # Emission order pipelines spans so the PE never waits on the
            # ACT psB->SBUF drains: scores(s+1) is issued before proj(s).
            # Diagonal-span tiles use partial-width matmuls (columns >= co
            # are the only causally valid ones).
            with tc.tile_pool(name="at", bufs=1) as at:
                wpsb = []
                for g in range(NG):
                    wt = at.tile([PD, C], F16, tag=f"wp{g}", name=f"wp{g}")
                    nc.sync.dma_start(out=wt, in_=wp_d[g])
                    wpsb.append(wt)

                def scores(s):
                    isl = slice(s * TCH, (s + 1) * TCH)
                    nj = 4 * (s + 1)
                    stact = []
                    for j in range(nj):
                        d = j - 4 * s
                        co = max(d, 0) * PD          # first valid i-column
                        psl = slice(s * TCH + co, (s + 1) * TCH)
                        ps = psA.tile([PD, TCH], F32, tag="ps_mm",
                                      name=f"pst{s}_{j}")
                        for m in range(NP):
                            nc.tensor.matmul(
                                ps[:, co:],
                                lhsT=qr8[m][:, :, j * PD:(j + 1) * PD],
                                rhs=kr8[m][:, :, psl],
                                start=(m == 0), stop=(m == NP - 1),
                                perf_mode=DR)
                        # softplus(x) = ln(1 + exp(x)); scores/sqrt(C) are
                        # bounded to a few units so exp cannot overflow
                        se = at.tile([PD, TCH], F32, tag="stexp", bufs=4,
                                     name=f"se{s}_{j}")
                        nc.scalar.activation(se[:, co:], ps[:, co:], AF.Exp,
                                             scale=INV_SQRT_C)
                        st = at.tile([PD, TCH], F16, tag="stact", bufs=20,
                                     name=f"st{s}_{j}")
                        nc.scalar.activation(st[:, co:], se[:, co:], AF.Ln,
                                             bias=1.0)
                        if d >= 0:
                            nc.vector.tensor_mul(st[:, co:], st[:, co:],
                                                 mskt[d][:, co:])
                        stact.append((st, co))
                    return stact

                def av(s, stact):
                    nj = 4 * (s + 1)
                    ot = []
                    for g in range(NG):
                        ps2 = psB.tile([PD, TCH], F32, tag="ps_ot",
                                       name=f"pot{s}_{g}")
                        for j in range(nj):
                            st, co = stact[j]
                            nc.tensor.matmul(
                                ps2[:, co:],
                                lhsT=vsb[j][:, g * PD:(g + 1) * PD],
                                rhs=st[:, co:],
                                start=(j == 0), stop=(j == nj - 1))
                        o = at.tile([PD, TCH], F16, tag="ot", bufs=17,
                                    name=f"ot{s}_{g}")
                        nc.scalar.activation(o, ps2, AF.Copy)
                        ot.append(o)
                    return ot

                def proj(s, ot):
                    for tt in range(4):
                        trow = s * TCH + tt * PD
                        for h in range(2):
                            ps = psA.tile([PD, TCH], F32, tag="ps_mm",
                                          name=f"ppr{s}_{tt}_{h}")
                            for g in range(NG):
                                nc.tensor.matmul(
                                    ps,
                                    lhsT=ot[g][:, tt * PD:(tt + 1) * PD],
                                    rhs=wpsb[g][:, h * TCH:(h + 1) * TCH],
                                    start=(g == 0), stop=(g == NG - 1))
                            # bp is added on the host; drain PSUM on whichever
                            # of ACT/DVE is free and DMA out
                            ob = at.tile([PD, TCH], F32, tag="ob", bufs=4,
                                         name=f"ob{s}_{tt}_{h}")
                            if h == 0:
                                nc.scalar.activation(ob, ps, AF.Copy)
                            else:
                                nc.vector.tensor_scalar_add(ob, ps, 0.0)
                            nc.sync.dma_start(
                                out=out_d[trow:trow + PD, h * TCH:(h + 1) * TCH],
                                in_=ob)

                prev_ot = None
                for s in range(NSP):
                    stact = scores(s)
                    if prev_ot is not None:
                        proj(s - 1, prev_ot)
                    prev_ot = av(s, stact)
                proj(NSP - 1, prev_ot)
    nc.finalize()
    return nc


def _static_tables():
    if "tables" in _CACHE:
        return _CACHE["tables"]
    perm = np.concatenate([np.arange(0, C, 2), np.arange(1, C, 2)])
    # reference computes angle[t, j] = t*j in FP32, then flips t; replicate
    # the fp32 rounding exactly, then evaluate cos/sin in f64 on that angle
    tfl = (T - 1 - np.arange(T)).astype(np.float32)
    jj = np.arange(H, dtype=np.float32)
    ang = (jj[:, None] * tfl[None, :]).astype(np.float32)  # [H, T]
    cosT = np.cos(ang.astype(np.float64)).astype(np.float16)
    sinT = np.sin(ang.astype(np.float64)).astype(np.float16)
    a = np.arange(PD)[:, None]
    b = np.arange(TCH)[None, :]
    masks = np.stack([(a + PD * d <= b) for d in range(NSP)])
    masks = masks.astype(np.float16)
    _CACHE["tables"] = (perm, cosT, sinT, masks)
    return _CACHE["tables"]


def prepare(x, Wk, bk, Wq, bq, Wv, bv, Wp, bp):
    """Build (cached) the Bass program and the per-core input maps."""
    x = np.asarray(x, dtype=np.float32)
    Wk, bk = np.asarray(Wk, np.float32), np.asarray(bk, np.float32)
    Wq, bq = np.asarray(Wq, np.float32), np.asarray(bq, np.float32)
    Wv, bv = np.asarray(Wv, np.float32), np.asarray(bv, np.float32)
    Wp, bp = np.asarray(Wp, np.float32), np.asarray(bp, np.float32)

    perm, cosT, sinT, masks = _static_tables()

    # wk8[e, i, p, mg, o] = Wk[mg*128+p, perm[g*128+o]] with g = [0,4,1,5,...][2e+i]
    worder = [0, 4, 1, 5, 2, 6, 3, 7]

    def w8(w):
        wp8 = np.ascontiguousarray(w[:, perm]).astype(ml_dtypes.float8_e4m3)
        byg = wp8.reshape(NG, PD, NG, PD).transpose(2, 1, 0, 3)  # [g, p, m, o]
        return np.ascontiguousarray(
            byg[worder].reshape(4, 2, PD, NG, PD).transpose(0, 2, 1, 3, 4))

    wk8 = w8(Wk)
    wq8 = w8(Wq)
    wv = np.ascontiguousarray(Wv.reshape(NG, PD, C)).astype(np.float16)
    wp = np.ascontiguousarray(Wp.reshape(NG, PD, C)).astype(np.float16)
    bkr = np.ascontiguousarray(bk[perm].reshape(NG, PD).T).astype(np.float32)
    bqr = np.ascontiguousarray(bq[perm].reshape(NG, PD).T).astype(np.float32)
    bvb = np.ascontiguousarray(np.broadcast_to(bv, (PD, C))).astype(np.float32)

    if "nc" not in _CACHE:
        _CACHE["nc"] = _build_nc()
    nc = _CACHE["nc"]

    shared = dict(wk8=wk8, wq8=wq8, wv=wv, wp=wp, bkr=bkr, bqr=bqr,
                  bvb=bvb, cosT=cosT, sinT=sinT, masks=masks)
    in_maps = []
    for i in range(NCORES):
        xT = np.ascontiguousarray(x[i].T)                      # [C, T] f32
        xt16 = xT.reshape(NG, PD, T).astype(np.float16)
        # xt8[ch, pp, p, i, t'] = xT[(2p+i)*128+pp, ch*512+t']
        xt8 = np.ascontiguousarray(
            xT.reshape(NP, 2, PD, NSP, TCH).transpose(3, 2, 0, 1, 4)
        ).astype(ml_dtypes.float8_e4m3)
        in_maps.append(dict(xt16=xt16, xt8=xt8, **shared))
    return nc, in_maps


def kernel(x, Wk, bk, Wq, bq, Wv, bv, Wp, bp):
    global LAST_RESULT
    nc, in_maps = prepare(x, Wk, bk, Wq, bq, Wv, bv, Wp, bp)
    res = run_bass_kernel_spmd(nc, in_maps, list(range(NCORES)))
    LAST_RESULT = res
    out = np.stack([res.results[i]["out"] for i in range(NCORES)], axis=0)
    # projection bias is applied on the host (exact in fp32)
    return out.astype(np.float32) + np.asarray(bp, np.float32)


# revision 24
# speedup vs baseline: 1.4097x; 1.4097x over previous
"""Trainium2 Bass kernel for nn_Attention_41996190220419.

Single-head causal attention with softplus weights and a time-flipped
rotary embedding, B=8 T=2048 C=1024 fp32.

Sharding: pure data-parallel over batch (1 batch element per NeuronCore,
8 cores, no collectives).

v2 over the 420us baseline:
  - K/Q projection GEMMs and the score GEMM run in fp8 e4m3 with
    MatmulPerfMode.DoubleRow (2 fp8 weights per PE cell, K=256 per
    instruction, ~1.4x bf16 throughput).  The V/AV/proj GEMMs stay
    16-bit: fp8 anywhere in the value path puts ~2.4% quantization
    error straight onto the output (softplus smoothing only protects
    the score path).  Simulated end-to-end L2 err 1.1e-2 vs 2e-2 gate.
  - x arrives host-pretransposed (fp16 for V, fp8 pairs for K/Q), so
    the PE transpose phase is gone.
  - 16-bit GEMMs use fp16 instead of bf16 (same PE speed, 8x lower
    rounding error).
  - rotary cos/sin tables are built from the fp32-rounded angle t*j the
    reference computes, eliminating a systematic trig mismatch.

Per-core phases:
  1. KT/QT = W^T x^T via fp8-DR (+bias via ACT), rotary rotation on DVE
     (fp16 cos/sin streamed), results quantized to fp8 pair tiles.
  2. V = x Wv + bv into resident fp16 SBUF tiles (overlaps the DVE
     rotation tail).
  3. per 512-wide i-span: ST[j,i] = sum_c QR[j,c] KR[i,c] (fp8-DR) for
     causal blocks, softplus = Ln(Exp(x)+1) on ACT, diagonal masks on
     DVE; OT[c,i] = sum_j V[j,c] ST[j,i] (fp16); OUT = OT^T Wp + bp.

The even/odd rotation pairs are turned into tile-level structure by
permuting the columns of Wk/Wq (and bk/bq) on the host to [evens|odds];
scores are invariant to any channel permutation applied to both K and Q.
"""

import os
import sys

if "/opt/trn_rl_repo" not in sys.path:
    sys.path.insert(0, "/opt/trn_rl_repo")

import numpy as np
import ml_dtypes

import concourse.bass as bass
import concourse.bacc as bacc
import concourse.mybir as mybir
import concourse.tile as tile
from concourse.bass_utils import run_bass_kernel_spmd

B, T, C = 8, 2048, 1024
H = C // 2
NCORES = 8
PD = 128
TCH = 512                 # t-chunk width (phase 1) == i-span width (attention)
NT = T // PD              # 16
NSP = T // TCH            # 4
NG = C // PD              # 8
NP = NG // 2              # 4 c-group pairs for DoubleRow
F16 = mybir.dt.float16
FP8 = mybir.dt.float8e4
F32 = mybir.dt.float32
AF = mybir.ActivationFunctionType
DR = mybir.MatmulPerfMode.DoubleRow
INV_SQRT_C = float(C) ** -0.5

_CACHE = {}

LAST_RESULT = None  # BassKernelResults of the most recent run (for profiling)


def _patch_act_tables():
    """Force every ACT func we use (Copy/Identity/Exp/Ln) to resolve to the
    single `natural_log_exp_and_others` table so the Exp/Ln alternation in
    the softplus does not thrash ACT_TABLE_LOADs (1.3us each)."""
    if _CACHE.get("act_patched"):
        return
    from concourse import hw_specs
    orig = hw_specs.get_activation_tables
    combined = "natural_log_exp_and_others"

    def patched(arch):
        tables = orig(arch)
        if combined in tables:
            keep = tables[combined]
            tables = {
                name: (s if name == combined else (s - keep))
                for name, s in tables.items()
            }
        return tables

    hw_specs.get_activation_tables = patched
    bacc.get_activation_tables = patched
    _CACHE["act_patched"] = True


def _build_nc():
    _patch_act_tables()
    nc = bacc.Bacc("TRN2", target_bir_lowering=False, debug=False,
                   num_devices=NCORES)

    xt16_d = nc.dram_tensor("xt16", [NG, PD, T], F16, kind="ExternalInput").ap()
    xt8_d = nc.dram_tensor("xt8", [NSP, PD, NP, 2, TCH], FP8,
                           kind="ExternalInput").ap()
    # g-axis pre-ordered [0,4,1,5,2,6,3,7] so slot pair 2e..2e+1 = groups e,e+4
    wk8_d = nc.dram_tensor("wk8", [4, PD, 2, NG, PD], FP8, kind="ExternalInput").ap()
    wq8_d = nc.dram_tensor("wq8", [4, PD, 2, NG, PD], FP8, kind="ExternalInput").ap()
    wv_d = nc.dram_tensor("wv", [NG, PD, C], F16, kind="ExternalInput").ap()
    wp_d = nc.dram_tensor("wp", [NG, PD, C], F16, kind="ExternalInput").ap()
    bkr_d = nc.dram_tensor("bkr", [PD, NG], F32, kind="ExternalInput").ap()
    bqr_d = nc.dram_tensor("bqr", [PD, NG], F32, kind="ExternalInput").ap()
    bvb_d = nc.dram_tensor("bvb", [PD, C], F32, kind="ExternalInput").ap()
    cos_d = nc.dram_tensor("cosT", [H, T], F16, kind="ExternalInput").ap()
    sin_d = nc.dram_tensor("sinT", [H, T], F16, kind="ExternalInput").ap()
    msk_d = nc.dram_tensor("masks", [NSP, PD, TCH], F16,
                           kind="ExternalInput").ap()
    out_d = nc.dram_tensor("out", [T, C], F32, kind="ExternalOutput").ap()

    with tile.TileContext(nc) as tc:
        with tc.tile_pool(name="persist", bufs=1) as pp, \
             tc.tile_pool(name="psA", bufs=4, space="PSUM") as psA, \
             tc.tile_pool(name="psB", bufs=4, space="PSUM") as psB:

            # fp8 rotated K/Q: pair tile P holds c-groups (2P, 2P+1)
            kr8 = [pp.tile([PD, 2, T], FP8, tag=f"kr{p}", name=f"kr{p}")
                   for p in range(NP)]
            qr8 = [pp.tile([PD, 2, T], FP8, tag=f"qr{p}", name=f"qr{p}")
                   for p in range(NP)]
            vsb = [pp.tile([PD, C], F16, tag=f"v{j}", name=f"v{j}")
                   for j in range(NT)]

            bkr = pp.tile([PD, NG], F32, name="bkr")
            nc.sync.dma_start(out=bkr, in_=bkr_d)
            bqr = pp.tile([PD, NG], F32, name="bqr")
            nc.sync.dma_start(out=bqr, in_=bqr_d)
            # bvb/masks DMAs are issued later (not needed until V GEMM /
            # attention) to keep the startup sync queue lean
            bvb = pp.tile([PD, C], F32, name="bvb")
            mskt = [pp.tile([PD, TCH], F16, tag=f"msk{d}", name=f"msk{d}")
                    for d in range(NSP)]

            # ---------------- phase 1+2: K/Q (rotated, fp8) then V --------
            with tc.tile_pool(name="p1", bufs=1) as p1:
                # DMA emission order == sync-queue order: first-needed first.
                xt8 = p1.tile([PD, NSP, NP, 2, TCH], FP8, name="xt8")
                wk8 = p1.tile([PD, 8, NG, PD], FP8, name="wk8")
                wq8 = p1.tile([PD, 8, NG, PD], FP8, name="wq8")
                nc.sync.dma_start(out=xt8[:, 0], in_=xt8_d[0])
                nc.sync.dma_start(out=wk8[:, 0:2], in_=wk8_d[0])
                for ch in range(1, NSP):
                    nc.sync.dma_start(out=xt8[:, ch], in_=xt8_d[ch])
                for e in range(1, 4):
                    nc.sync.dma_start(out=wk8[:, 2 * e:2 * e + 2],
                                      in_=wk8_d[e])

                W2 = 2 * TCH     # rotation operates on 1024-wide halves

                def kq_gemms(wname, w8t, brt, dst):
                    # weight slot for group g: pairs (e, e+4) live at 2e, 2e+1
                    for e in range(4):
                        trig = {}
                        for c2 in range(2):
                            w2sl = slice(c2 * W2, (c2 + 1) * W2)
                            cs = p1.tile([PD, W2], F16, tag="trig", bufs=8,
                                         name=f"cs{wname}{e}_{c2}")
                            nc.sync.dma_start(
                                out=cs, in_=cos_d[e * PD:(e + 1) * PD, w2sl])
                            sn = p1.tile([PD, W2], F16, tag="trig", bufs=8,
                                         name=f"sn{wname}{e}_{c2}")
                            nc.sync.dma_start(
                                out=sn, in_=sin_d[e * PD:(e + 1) * PD, w2sl])
                            trig[c2] = (cs, sn)
                        for c2 in range(2):
                            w2sl = slice(c2 * W2, (c2 + 1) * W2)
                            tmp = {}
                            for i, g in enumerate((e, e + 4)):
                                kt = p1.tile([PD, W2], F16, tag="kttmp",
                                             bufs=6, name=f"kt{wname}{g}_{c2}")
                                for hh in range(2):
                                    ch = 2 * c2 + hh
                                    hsl = slice(hh * TCH, (hh + 1) * TCH)
                                    ps = psA.tile([PD, TCH], F32, tag="ps_mm",
                                                  name=f"pkq{wname}{g}_{ch}")
                                    for m in range(NP):
                                        nc.tensor.matmul(
                                            ps,
                                            lhsT=w8t[:, 2 * e + i, 2 * m:2 * m + 2, :],
                                            rhs=xt8[:, ch, m],
                                            start=(m == 0), stop=(m == NP - 1),
                                            perf_mode=DR)
                                    nc.scalar.activation(kt[:, hsl], ps,
                                                         AF.Identity,
                                                         bias=brt[:, g:g + 1])
                                tmp[g] = kt
                            cs, sn = trig[c2]
                            ze, zo = tmp[e], tmp[e + 4]
                            # r_even = cos*z_e + sin*z_o ; r_odd = cos*z_o - sin*z_e
                            # all DVE ops stay 16-bit (2x mode); the fp8
                            # quantize runs on ACT which has slack here
                            t1 = p1.tile([PD, W2], F16, tag="rot", bufs=8,
                                         name=f"r1{wname}{e}_{c2}")
                            nc.vector.tensor_mul(t1, ze, cs)
                            t2 = p1.tile([PD, W2], F16, tag="rot", bufs=8,
                                         name=f"r2{wname}{e}_{c2}")
                            nc.vector.tensor_mul(t2, zo, sn)
                            r0 = p1.tile([PD, W2], F16, tag="rot", bufs=8,
                                         name=f"r0{wname}{e}_{c2}")
                            nc.vector.tensor_add(r0, t1, t2)
                            nc.scalar.activation(dst[e // 2][:, e % 2, w2sl],
                                                 r0, AF.Copy)
                            t3 = p1.tile([PD, W2], F16, tag="rot", bufs=8,
                                         name=f"r3{wname}{e}_{c2}")
                            nc.vector.tensor_mul(t3, zo, cs)
                            t4 = p1.tile([PD, W2], F16, tag="rot", bufs=8,
                                         name=f"r4{wname}{e}_{c2}")
                            nc.vector.tensor_mul(t4, ze, sn)
                            r1 = p1.tile([PD, W2], F16, tag="rot", bufs=8,
                                         name=f"r1o{wname}{e}_{c2}")
                            nc.vector.tensor_sub(r1, t3, t4)
                            nc.scalar.activation(dst[2 + e // 2][:, e % 2, w2sl],
                                                 r1, AF.Copy)

                kq_gemms("k", wk8, bkr, kr8)

                # V-GEMM operands + Q weights stream behind the K-phase DMAs
                xt16 = []
                for g in range(NG):
                    t16 = p1.tile([PD, T], F16, tag=f"xt16_{g}",
                                  name=f"xt16_{g}")
                    nc.sync.dma_start(out=t16, in_=xt16_d[g])
                    xt16.append(t16)
                wvsb = []
                for g in range(NG):
                    wt = p1.tile([PD, C], F16, tag="wv", bufs=8,
                                 name=f"wv{g}")
                    nc.sync.dma_start(out=wt, in_=wv_d[g])
                    wvsb.append(wt)
                for e in range(4):
                    nc.sync.dma_start(out=wq8[:, 2 * e:2 * e + 2],
                                      in_=wq8_d[e])

                nc.sync.dma_start(out=bvb, in_=bvb_d)
                for d in range(NSP):
                    nc.sync.dma_start(out=mskt[d], in_=msk_d[d])

                kq_gemms("q", wq8, bqr, qr8)

                # V GEMM (fp16): writes into resident vsb tiles
                for tt in range(NT):
                    for h in range(2):
                        ps = psB.tile([PD, TCH], F32, tag="ps_ot",
                                      name=f"pv{tt}_{h}")
                        for g in range(NG):
                            nc.tensor.matmul(
                                ps,
                                lhsT=xt16[g][:, tt * PD:(tt + 1) * PD],
                                rhs=wvsb[g][:, h * TCH:(h + 1) * TCH],
                                start=(g == 0), stop=(g == NG - 1))
                        nc.vector.tensor_add(vsb[tt][:, h * TCH:(h + 1) * TCH],
                                             ps, bvb[:, h * TCH:(h + 1) * TCH])

            # ---------------- phase 3: attention + projection -------------
            # Emission order pipelines spans so the PE never waits on the
            # ACT psB->SBUF drains: scores(s+1) is issued before proj(s).
            # Diagonal-span tiles use partial-width matmuls (columns >= co
            # are the only causally valid ones).
            with tc.tile_pool(name="at", bufs=1) as at:
                wpsb = []
                for g in range(NG):
                    wt = at.tile([PD, C], F16, tag=f"wp{g}", name=f"wp{g}")
                    nc.sync.dma_start(out=wt, in_=wp_d[g])
                    wpsb.append(wt)

                def scores(s):
                    isl = slice(s * TCH, (s + 1) * TCH)
                    nj = 4 * (s + 1)
                    stact = []
                    for j in range(nj):
                        d = j - 4 * s
                        co = max(d, 0) * PD          # first valid i-column
                        psl = slice(s * TCH + co, (s + 1) * TCH)
                        ps = psA.tile([PD, TCH], F32, tag="ps_mm",
                                      name=f"pst{s}_{j}")
                        for m in range(NP):
                            nc.tensor.matmul(
                                ps[:, co:],
                                lhsT=qr8[m][:, :, j * PD:(j + 1) * PD],
                                rhs=kr8[m][:, :, psl],
                                start=(m == 0), stop=(m == NP - 1),
                                perf_mode=DR)
                        # softplus(x) = ln(1 + exp(x)); scores/sqrt(C) are
                        # bounded to a few units so exp cannot overflow
                        se = at.tile([PD, TCH], F32, tag="stexp", bufs=4,
                                     name=f"se{s}_{j}")
                        nc.scalar.activation(se[:, co:], ps[:, co:], AF.Exp,
                                             scale=INV_SQRT_C)
                        st = at.tile([PD, TCH], F16, tag="stact", bufs=20,
                                     name=f"st{s}_{j}")
                        nc.scalar.activation(st[:, co:], se[:, co:], AF.Ln,
                                             bias=1.0)
                        if d >= 0:
                            nc.vector.tensor_mul(st[:, co:], st[:, co:],
                                                 mskt[d][:, co:])
                        stact.append((st, co))
                    return stact

                def av(s, stact):
                    nj = 4 * (s + 1)
                    ot = []
                    for g in range(NG):
                        ps2 = psB.tile([PD, TCH], F32, tag="ps_ot",
                                       name=f"pot{s}_{g}")
                        for j in range(nj):
                            st, co = stact[j]
                            nc.tensor.matmul(
                                ps2[:, co:],
                                lhsT=vsb[j][:, g * PD:(g + 1) * PD],
                                rhs=st[:, co:],
                                start=(j == 0), stop=(j == nj - 1))
                        o = at.tile([PD, TCH], F16, tag="ot", bufs=17,
                                    name=f"ot{s}_{g}")
                        nc.scalar.activation(o, ps2, AF.Copy)
                        ot.append(o)
                    return ot

                def proj(s, ot):
                    for tt in range(4):
                        trow = s * TCH + tt * PD
                        for h in range(2):
                            ps = psA.tile([PD, TCH], F32, tag="ps_mm",
                                          name=f"ppr{s}_{tt}_{h}")
                            for g in range(NG):
                                nc.tensor.matmul(
                                    ps,
                                    lhsT=ot[g][:, tt * PD:(tt + 1) * PD],
                                    rhs=wpsb[g][:, h * TCH:(h + 1) * TCH],
                                    start=(g == 0), stop=(g == NG - 1))
                            # bp is added on the host; drain PSUM on whichever
                            # of ACT/DVE is free and DMA out
                            ob = at.tile([PD, TCH], F32, tag="ob", bufs=4,
                                         name=f"ob{s}_{tt}_{h}")
                            if h == 0:
                                nc.scalar.activation(ob, ps, AF.Copy)
                            else:
                                nc.vector.tensor_scalar_add(ob, ps, 0.0)
                            nc.sync.dma_start(
                                out=out_d[trow:trow + PD, h * TCH:(h + 1) * TCH],
                                in_=ob)

                prev_ot = None
                for s in range(NSP):
                    stact = scores(s)
                    if prev_ot is not None:
                        proj(s - 1, prev_ot)
                    prev_ot = av(s, stact)
                proj(NSP - 1, prev_ot)
    nc.finalize()
    return nc


def _static_tables():
    if "tables" in _CACHE:
        return _CACHE["tables"]
    perm = np.concatenate([np.arange(0, C, 2), np.arange(1, C, 2)])
    # reference computes angle[t, j] = t*j in FP32, then flips t; replicate
    # the fp32 rounding exactly, then evaluate cos/sin in f64 on that angle
    tfl = (T - 1 - np.arange(T)).astype(np.float32)
    jj = np.arange(H, dtype=np.float32)
    ang = (jj[:, None] * tfl[None, :]).astype(np.float32)  # [H, T]
    cosT = np.cos(ang.astype(np.float64)).astype(np.float16)
    sinT = np.sin(ang.astype(np.float64)).astype(np.float16)
    a = np.arange(PD)[:, None]
    b = np.arange(TCH)[None, :]
    masks = np.stack([(a + PD * d <= b) for d in range(NSP)])
    masks = masks.astype(np.float16)
    _CACHE["tables"] = (perm, cosT, sinT, masks)
    return _CACHE["tables"]


def prepare(x, Wk, bk, Wq, bq, Wv, bv, Wp, bp):
    """Build (cached) the Bass program and the per-core input maps."""
    x = np.asarray(x, dtype=np.float32)
    Wk, bk = np.asarray(Wk, np.float32), np.asarray(bk, np.float32)
    Wq, bq = np.asarray(Wq, np.float32), np.asarray(bq, np.float32)
    Wv, bv = np.asarray(Wv, np.float32), np.asarray(bv, np.float32)
    Wp, bp = np.asarray(Wp, np.float32), np.asarray(bp, np.float32)

    perm, cosT, sinT, masks = _static_tables()

    # wk8[e, i, p, mg, o] = Wk[mg*128+p, perm[g*128+o]] with g = [0,4,1,5,...][2e+i]
    worder = [0, 4, 1, 5, 2, 6, 3, 7]

    def w8(w):
        wp8 = np.ascontiguousarray(w[:, perm]).astype(ml_dtypes.float8_e4m3)
        byg = wp8.reshape(NG, PD, NG, PD).transpose(2, 1, 0, 3)  # [g, p, m, o]
        return np.ascontiguousarray(
            byg[worder].reshape(4, 2, PD, NG, PD).transpose(0, 2, 1, 3, 4))

    wk8 = w8(Wk)
    wq8 = w8(Wq)
    wv = np.ascontiguousarray(Wv.reshape(NG, PD, C)).astype(np.float16)
    wp = np.ascontiguousarray(Wp.reshape(NG, PD, C)).astype(np.float16)
    bkr = np.ascontiguousarray(bk[perm].reshape(NG, PD).T).astype(np.float32)
    bqr = np.ascontiguousarray(bq[perm].reshape(NG, PD).T).astype(np.float32)
    bvb = np.ascontiguousarray(np.broadcast_to(bv, (PD, C))).astype(np.float32)

    if "nc" not in _CACHE:
        _CACHE["nc"] = _build_nc()
    nc = _CACHE["nc"]

    shared = dict(wk8=wk8, wq8=wq8, wv=wv, wp=wp, bkr=bkr, bqr=bqr,
                  bvb=bvb, cosT=cosT, sinT=sinT, masks=masks)
    in_maps = []
    for i in range(NCORES):
        xT = np.ascontiguousarray(x[i].T)                      # [C, T] f32
        xt16 = xT.reshape(NG, PD, T).astype(np.float16)
        # xt8[ch, pp, p, i, t'] = xT[(2p+i)*128+pp, ch*512+t']
        xt8 = np.ascontiguousarray(
            xT.reshape(NP, 2, PD, NSP, TCH).transpose(3, 2, 0, 1, 4)
        ).astype(ml_dtypes.float8_e4m3)
        in_maps.append(dict(xt16=xt16, xt8=xt8, **shared))
    return nc, in_maps


def kernel(x, Wk, bk, Wq, bq, Wv, bv, Wp, bp):
    global LAST_RESULT
    nc, in_maps = prepare(x, Wk, bk, Wq, bq, Wv, bv, Wp, bp)
    res = run_bass_kernel_spmd(nc, in_maps, list(range(NCORES)))
    LAST_RESULT = res
    out = np.stack([res.results[i]["out"] for i in range(NCORES)], axis=0)
    # projection bias is applied on the host (exact in fp32)
    return out.astype(np.float32) + np.asarray(bp, np.float32)
